# revision 1
# baseline (speedup 1.0000x reference)
"""Trainium2 Bass kernel for nn_NPairsLoss (N-pairs loss over n=4096 rows).

Reference math (X = inputs.reshape(4096, 512), prod = X @ X.T, class/part row
masks): loss = (1/n) * sum_i [2*sum_{sadc_i} g_ij + sum_{dasc_i} g_ij] with
g_ij = log1p(S_i exp(-prod_ij)) and S_i the exp-sum over diff-class/diff-part
columns j.

Decomposition (exact to ~1e-7 relative):
    g_ij = ln(S_i) - prod_ij + e_ij/S_i,  e_ij = exp(prod_ij)
so every masked g-sum splits into count*ln(S), a masked *linear* prod sum
(= x_i . sum-vector, host GEMV), and a masked exp sum / S. The same-class
masked exp sums (Ec, Ecp) have support only on the ~32x32 per-class Gram
blocks (0.8% of the matrix) and are computed exactly on the host from tiny
per-class Grams of the same fp8-cast X the device multiplies.

DEVICE (8 cores, SPMD; rows sorted by (part, class), 512 rows/core, per-core
column order [own rows | rest of own part | other parts] so cols [0:1024) are
always the same-part region): the full n^2 work -
  fp8 DoubleRow GEMM (K=512 as [128 partitions x 4 subtiles], N=512 per
  matmul, fp32 PSUM) -> per 1024-col PSUM tile: ACT exp with fused fp32
  row-sum accum. Output per core: (128, 16) f32 = per (block k, tile q)
  row sums of exp; tile q=0 of each block is the same-part sum Ep.
HOST: S = Eall - Ep - Ec + Ecp, Ls = ln S, and the weighted assembly
    w = 2*(1024*Ls - Pq + Ep/S) + (4bc*Ls - Mp + Ec/S) - 3*(bc*Ls - Mpq + Ecp/S)
    loss = sum(w)/n   (bc = batch count of the row's class).
"""
import os
from contextlib import ExitStack

import numpy as np
import ml_dtypes

import concourse.bass as bass
import concourse.tile as tile
from concourse import bacc, mybir
from concourse import bass_utils

B, P, D, C = 1024, 4, 512, 128
N = B * P                      # 4096 rows
NCORES = 8
RPC = N // NCORES              # 512 rows per core
BLK = 128                      # rows per block (SBUF partitions)
NBLK = RPC // BLK              # 4 blocks per core
QUARTER = 2 * RPC              # 1024 cols = same-part region

FP8 = mybir.dt.float8e4
F32 = mybir.dt.float32
nfp8 = ml_dtypes.float8_e4m3fn

_CACHE = {}


def _build_nc():
    nc = bacc.Bacc(
        "TRN2",
        target_bir_lowering=False,
        debug=False,
        enable_asserts=False,
        num_devices=NCORES,
    )
    xt_d = nc.dram_tensor("xt", [BLK, 4 * N], FP8, kind="ExternalInput")
    zc_d = nc.dram_tensor("zc", [BLK, 1], F32, kind="ExternalInput")
    out_d = nc.dram_tensor("out", [BLK, 12], F32, kind="ExternalOutput")

    AF = mybir.ActivationFunctionType
    # psum tiles per block: [0:1024] (quarter/Ep), [1024:3072], [3072:4096]
    TILES = [(0, 1024, "edge"), (1024, 3072, "mid"), (3072, 4096, "edge")]

    with tile.TileContext(nc) as tc, ExitStack() as ctx:
        const = ctx.enter_context(tc.tile_pool(name="const", bufs=1))
        psum = ctx.enter_context(tc.tile_pool(name="psum", bufs=1, space="PSUM"))
        sink = ctx.enter_context(tc.tile_pool(name="sink", bufs=2))

        # zeros for the activation bias (DMA'd so no Pool/DVE memset is needed
        # and the exit barrier spans only SP/PE/ACT)
        zc_t = const.tile([BLK, 1], F32, tag="zc")
        nc.sync.dma_start(zc_t[:], zc_d.ap())
        # xt is X^T in fp8, stored as [128, s*4096 + c] with contraction index
        # d = s*128 + p (s = subtile 0..3, p = partition). DMA col-ascending,
        # fine-grained early pieces so the first GEMMs start ASAP.
        xtall = const.tile([BLK, 4 * N], FP8, tag="xtall")
        PIECES = [(0, 512), (512, 1024), (1024, 2048), (2048, 3072), (3072, 4096)]
        for plo, phi in PIECES:
            for s in range(4):
                lo = s * N + plo
                hi = s * N + phi
                nc.sync.dma_start(xtall[:, lo:hi], xt_d.ap()[:, lo:hi])
        xt3 = xtall[:].rearrange("p (s c) -> p s c", s=4)
        out_t = const.tile([BLK, 12], F32, tag="out")

        for k in range(NBLK):
            for q, (tlo, thi, tag) in enumerate(TILES):
                w = thi - tlo
                ps = psum.tile([BLK, w], F32, tag=tag, bufs=(2 if tag == "edge" else 1),
                               name=f"ps{k}_{q}")
                for sp in range(2):          # s-subtile pairs (DoubleRow K=256)
                    for half in range(w // 512):
                        j = (tlo // 512) + half
                        nc.tensor.matmul(
                            ps[:, half * 512:(half + 1) * 512],
                            xt3[:, 2 * sp:2 * sp + 2, k * BLK:(k + 1) * BLK],
                            xt3[:, 2 * sp:2 * sp + 2, j * 512:(j + 1) * 512],
                            start=(sp == 0),
                            stop=(sp == 1),
                            perf_mode=mybir.MatmulPerfMode.DoubleRow,
                        )
                # exp(prod) with fused fp32 row-sum -> out col; the bf16
                # exp values themselves are dead (sink tile)
                e_sink = sink.tile([BLK, w], mybir.dt.bfloat16, tag=f"es_{tag}",
                                   name=f"es{k}_{q}")
                nc.scalar.activation(
                    e_sink[:], ps[:], AF.Exp, bias=zc_t[:],
                    accum_out=out_t[:, 3 * k + q:3 * k + q + 1],
                )

        nc.sync.dma_start(out_d.ap(), out_t[:])

    nc.compile()
    return nc


def host_prep(inputs, targets):
    """Per-core device inputs + host-side aux for the combine step."""
    X = np.ascontiguousarray(np.asarray(inputs, dtype=np.float32).reshape(N, D))
    tg = np.asarray(targets).astype(np.int64)
    t = np.repeat(tg, P)
    part = np.tile(np.arange(P, dtype=np.int64), B)
    order = np.lexsort((t, part))
    X_s = X[order]
    t_s = t[order]
    X8 = X_s.astype(nfp8)
    in_maps = []
    for c in range(NCORES):
        p = c // 2
        own = np.arange(RPC * c, RPC * (c + 1))
        buddy = np.arange(RPC * (c ^ 1), RPC * ((c ^ 1) + 1))
        lo, hi = QUARTER * p, QUARTER * (p + 1)
        rest = np.concatenate([np.arange(0, lo), np.arange(hi, N)])
        colperm = np.concatenate([own, buddy, rest])
        # (D, N) -> (4, 128, N) -> (128, 4, N) -> (128, 4N); d = s*128 + p
        xtT = X8[colperm].T                                          # (D, N)
        xt = np.ascontiguousarray(
            xtT.reshape(4, BLK, N).transpose(1, 0, 2).reshape(BLK, 4 * N)
        )
        in_maps.append({"xt": xt, "zc": np.zeros((BLK, 1), np.float32)})
    aux = dict(Xb=X8.astype(np.float64), t_s=t_s, tg=tg)
    return in_maps, aux


def host_combine(outs, aux):
    Xb, t_s, tg = aux["Xb"], aux["t_s"], aux["tg"]
    part_s = np.repeat(np.arange(P), B)
    bc = np.bincount(tg, minlength=C)
    # linear prod sum vectors
    qsum = np.stack([Xb[part_s == p].sum(axis=0) for p in range(P)])
    onehot = np.zeros((N, C))
    onehot[np.arange(N), t_s] = 1.0
    clssum = onehot.T @ Xb
    cpsum = np.stack([onehot[part_s == p].T @ Xb[part_s == p] for p in range(P)])
    # exact same-class masked exp sums via per-class Grams (~32x32 each)
    Ec = np.zeros(N)
    Ecp = np.zeros(N)
    for c in range(C):
        rows_c = np.nonzero(t_s == c)[0]
        if len(rows_c) == 0:
            continue
        V = Xb[rows_c]
        E = np.exp(V @ V.T)
        Ec[rows_c] = E.sum(axis=1)
        pc = part_s[rows_c]
        for p in range(P):
            m = pc == p
            if m.any():
                Ecp[rows_c[m]] = E[np.ix_(m, m)].sum(axis=1)
    total = 0.0
    for ci, o in enumerate(outs):
        o = np.asarray(o, np.float64)
        rows = np.arange(RPC * ci, RPC * (ci + 1))
        p = ci // 2
        x = Xb[rows]
        cls = t_s[rows]
        Pq = x @ qsum[p]
        Mp = (x * clssum[cls]).sum(axis=1)
        Mpq = (x * cpsum[p][cls]).sum(axis=1)
        cnt_c = 4.0 * bc[cls]
        cnt_cp = 1.0 * bc[cls]
        for k in range(NBLK):
            r = slice(BLK * k, BLK * (k + 1))
            rr = rows[r]
            Ep = o[:, 3 * k + 0]
            Eall = o[:, 3 * k:3 * k + 3].sum(axis=1)
            S = Eall - Ep - Ec[rr] + Ecp[rr]
            Ls = np.log(S)
            Gp_ = 1024.0 * Ls - Pq[r] + Ep / S
            Gc_ = cnt_c[r] * Ls - Mp[r] + Ec[rr] / S
            Gcp_ = cnt_cp[r] * Ls - Mpq[r] + Ecp[rr] / S
            total += float((2.0 * Gp_ + Gc_ - 3.0 * Gcp_).sum())
    return np.float32(total / N)


def kernel(inputs, targets):
    if "nc" not in _CACHE:
        _CACHE["nc"] = _build_nc()
    nc = _CACHE["nc"]
    in_maps, aux = host_prep(inputs, targets)
    kwargs = {}
    if bool(int(os.environ.get("NPAIRS_TRACE", "0"))):
        kwargs = dict(trace=True, tmpdir=os.environ.get("NPAIRS_TMPDIR") or None)
    res = bass_utils.run_bass_kernel_spmd(
        nc, in_maps, core_ids=list(range(NCORES)), **kwargs
    )
    _CACHE["last_results"] = res
    outs = [r["out"] for r in res.results]
    return host_combine(outs, aux)

